# revision 8
# baseline (speedup 1.0000x reference)
"""Trainium2 Bass kernel for nn_EyetrackRegressionBiLSTM.

Strategy: data-parallel over batch B=128 across 8 NeuronCores (b=16 each).
Each core runs both LSTM layers on its batch shard, chunk-phased over T:
  GEMM0 (input projection, fp32r on PE) -> L0 scan -> GEMM1 -> L1 scan -> head.
Feature-major layout throughout: gates live as 8 partition-tiles of [128, b]
packed into 4 PSUM banks (one per PE row-strip).  Sigmoid is computed via
tanh (sigmoid and tanh never share an ACT table): sigma(x) = 0.5*(1+tanh(x/2)),
with the 0.5 pre-scales folded into the weights on the host and the cell state
stored as c~ = 2c.
"""
import os
from contextlib import ExitStack

import numpy as np

import concourse.bass as bass
import concourse.bacc as bacc
import concourse.tile as tile
import concourse.mybir as mybir
from concourse.bass_utils import run_bass_kernel_spmd

F32 = mybir.dt.float32
F32R = mybir.dt.float32r
BF16 = mybir.dt.float16
AOP = mybir.AluOpType
AF = mybir.ActivationFunctionType

IN, H, PJ, T, B, G = 300, 256, 6, 512, 128, 1024
NCORE = 8
BS = B // NCORE          # 16 batch per core
TC = 64                  # timesteps per chunk
NCH = T // TC
C16 = TC * BS            # cols per chunk slab (per gate tile) = 1024
TB = T * BS              # 8192

# gate-row tile order [i0,i1,f0,f1,o0,o1,g0,g1]; original [i,f,g,o]
_TILE_ORDER = [0, 1, 2, 3, 6, 7, 4, 5]
PERM = np.concatenate([np.arange(128 * t, 128 * t + 128) for t in _TILE_ORDER])
SCALE = np.ones((G, 1), np.float32)
SCALE[:768] = 0.5        # i,f,o rows (first 6 tiles) pre-scaled for tanh-trick


def _strip_img(w6):
    """w6: [G, 6] (PERM-ordered, scaled).  -> [128,128] PE strip image:
    rows 32s+6j+r = tile(2s+j) row r (transposed)."""
    img = np.zeros((128, 128), np.float32)
    for tl in range(8):
        s, j = tl // 2, tl % 2
        img[32 * s + 6 * j: 32 * s + 6 * j + 6, :] = w6[128 * tl:128 * tl + 128, :].T
    return img


def _proj_pack(whr, sub):
    """whr: [6, H] (x0.5 folded).  -> [2,128,128]: chunk kc rows=H-slice,
    cols 32s+6*sub..+6 = whr columns; else zero."""
    out = np.zeros((2, 128, 128), np.float32)
    for kc in range(2):
        blk = whr[:, 128 * kc:128 * kc + 128].T      # [128, 6]
        for s in range(4):
            out[kc, :, 32 * s + 6 * sub: 32 * s + 6 * sub + 6] = blk
    return out


def build_kernel(T_=T, TC_=TC):
    """Build the Bass module (one core's SPMD program). Returns (nc, names)."""
    nch = T_ // TC_
    c16 = TC_ * BS
    tb = T_ * BS
    nc = bacc.Bacc("TRN2", target_bir_lowering=False, debug=False,
                   enable_asserts=False, num_devices=NCORE)

    dram = {}
    def din(name, shape, dt=F32):
        dram[name] = nc.dram_tensor(name, shape, dt, kind="ExternalInput").ap()
    def dout(name, shape, dt=F32):
        dram[name] = nc.dram_tensor(name, shape, dt, kind="ExternalOutput").ap()

    din("xT", [128, 3 * tb], BF16)            # GEMM0 rhs: [128,(c:3),(t),(b)]
    din("w0ihT", [128, 3 * 1024], BF16)       # GEMM0 lhsT: [128,(c:3),(gtile:8),(gcol:128)]
    din("w0img", [128, 128], BF16)      # L0 recurrent strip image
    din("w1yimg", [128, 128], BF16)     # L1 input(y0) strip image
    din("w1himg", [128, 128], BF16)     # L1 recurrent strip image
    din("wp0a", [128, 256], BF16)       # L0 proj lhsT (A-pattern), kc-major
    din("wp0b", [128, 256], BF16)
    din("wp1a", [128, 256], BF16)
    din("wp1b", [128, 256], BF16)
    din("b1img", [128, 8])              # L1 bias per gate tile (ACT bias vecs)
    din("headT", [8, tb])               # rows 0-5 mask6, rows 6-7 feature
    din("wl8", [8, 1])                  # head lhsT (rows 0-5 Wl[2:8], 6-7 Wl[0:2])
    din("blv", [1, 1])                  # head bias
    dout("ret6", [6, tb])
    dout("pred", [1, tb])
    if os.environ.get("KDEBUG"):
        dout("dbg_xg0", [128, 8 * c16])
        dout("dbg_xg1", [128, 8 * c16])
        dout("dbg_bufAB", [128, (TC_ + 1) * 32])
        dout("dbg_bufHAB", [128, (TC_ + 1) * 32])
        dout("dbg_cs0", [128, 32])
        dout("dbg_cs1", [128, 32])

    with tile.TileContext(nc, trace_sim=False) as tc:
        with ExitStack() as ctx:
            _build(ctx, tc, dram, T_, TC_, nch, c16)

    nc.compile()
    return nc, dram


def _build(ctx, tc, d, T_, TC_, nch, c16):
    nc = tc.nc
    const = ctx.enter_context(tc.tile_pool(name="const", bufs=1))
    work = ctx.enter_context(tc.tile_pool(name="work", bufs=2))
    ps_g = ctx.enter_context(tc.tile_pool(name="psg", bufs=1, space="PSUM"))
    ps_p = ctx.enter_context(tc.tile_pool(name="psp", bufs=1, space="PSUM"))
    ps_m = ctx.enter_context(tc.tile_pool(name="psm", bufs=2, space="PSUM"))

    # ---- persistent SBUF ----
    w0ihT = const.tile([128, 3 * 1024], BF16, tag="w0ihT")
    w0img = const.tile([128, 128], BF16, tag="w0img")
    w1yimg = const.tile([128, 128], BF16, tag="w1yimg")
    w1himg = const.tile([128, 128], BF16, tag="w1himg")
    wp = {}
    for nm in ("wp0a", "wp0b", "wp1a", "wp1b"):
        wp[nm] = const.tile([128, 256], BF16, tag=nm, name=nm)
    b1img = const.tile([128, 8], F32, tag="b1img")
    wl8 = const.tile([8, 1], F32, tag="wl8")
    blv = const.tile([1, 1], F32, tag="blv")
    for nm, t_ in [("w0ihT", w0ihT), ("w0img", w0img), ("w1yimg", w1yimg),
                   ("w1himg", w1himg), ("b1img", b1img), ("wl8", wl8),
                   ("blv", blv)] + [(k, v) for k, v in wp.items()]:
        nc.sync.dma_start(t_[:], d[nm][:])

    xg0 = [const.tile([128, 8 * c16], F32, tag=f"xg0_{i}", name=f"xg0_{i}") for i in range(2)]
    xg1 = const.tile([128, 8 * c16], F32, tag="xg1")
    xTc = [const.tile([128, 3 * c16], BF16, tag=f"xTc_{i}", name=f"xTc_{i}") for i in range(2)]
    bufAB = const.tile([128, (TC_ + 1) * 32], BF16, tag="bufAB")
    bufHAB = const.tile([128, (TC_ + 1) * 32], BF16, tag="bufHAB")
    cs0 = const.tile([128, 32], F32, tag="cs0")
    cs1 = const.tile([128, 32], F32, tag="cs1")
    headTc = const.tile([8, c16], F32, tag="headTc")
    predb = const.tile([1, c16], F32, tag="predb")

    gpsum = ps_g.tile([128, 2048], F32, tag="gates")   # 4 banks, strip s -> bank s
    ppA = ps_p.tile([128, 16], F32, tag="projA")       # one bank each: A/B
    ppB = ps_p.tile([128, 16], F32, tag="projB")

    nc.vector.memset(bufAB[:, 0:32], 0.0)
    nc.vector.memset(bufHAB[:, 0:32], 0.0)
    nc.vector.memset(cs0[:], 0.0)
    nc.vector.memset(cs1[:], 0.0)

    def gps_ap():
        return gpsum[:].rearrange("p (s c) -> p s c", s=4)[:, :, 0:32]

    def load_chunk(k):
        kb = k % 2
        for c in range(3):
            nc.sync.dma_start(xTc[kb][:, c * c16:(c + 1) * c16],
                              d["xT"][:, c * (T_ * BS) + k * c16:
                                         c * (T_ * BS) + (k + 1) * c16])

    def gemm0_groups(k):
        """Yield thunks: one (gtile, nhalf) psum group each (emit-interleaved)."""
        kb = k % 2
        nh_sz = 512
        nhn = c16 // nh_sz if c16 >= nh_sz else 1
        nh_sz = min(nh_sz, c16)
        for gt in range(8):
            for nh in range(nhn):
                def go(gt=gt, nh=nh):
                    ps = ps_m.tile([128, 512], F32, tag="gps")
                    for c in range(3):
                        nc.tensor.matmul(
                            ps[:, 0:nh_sz],
                            w0ihT[:, c * 1024 + gt * 128: c * 1024 + (gt + 1) * 128],
                            xTc[kb][:, c * c16 + nh * nh_sz: c * c16 + (nh + 1) * nh_sz],
                            start=(c == 0), stop=(c == 2), tile_position=(0, 0))
                    # scatter-copy psum -> xg0 slab [(t),(tile:16cols)]
                    o = xg0[kb][:].rearrange("p (t q) -> p t q", q=128)
                    nsteps = nh_sz // BS
                    nc.scalar.activation(
                        o[:, nh * nsteps:(nh + 1) * nsteps,
                          16 * gt:16 * gt + 16],
                        ps[:, 0:nh_sz].rearrange("p (t b) -> p t b", b=BS),
                        AF.Copy)
                yield go

    def gemm1(k):
        for gt in range(8):
            s, j = gt // 2, gt % 2
            for nh in range(c16 // 512 if c16 >= 512 else 1):
                nh_sz = min(512, c16)
                nst = nh_sz // BS
                ps = ps_m.tile([128, 512], F32, tag="gps")
                rhs = bufAB[:].rearrange("p (t q) -> p t q", q=32)
                if j == 0:
                    rr = rhs[32 * s:32 * s + 12, 1 + nh * nst:1 + (nh + 1) * nst, 0:16]
                    lh = w1yimg[32 * s:32 * s + 12, :]
                else:
                    rr = rhs[32 * s:32 * s + 12, 1 + nh * nst:1 + (nh + 1) * nst, 16:32]
                    lh = w1yimg[32 * s:32 * s + 12, :]
                nc.tensor.matmul(ps[:, 0:nh_sz], lh, rr,
                                 start=True, stop=True, tile_position=(32 * s, 0))
                o = xg1[:].rearrange("p (t q) -> p t q", q=128)
                nc.scalar.activation(
                    o[:, nh * nst:(nh + 1) * nst, 16 * gt:16 * gt + 16],
                    ps[:, 0:nh_sz].rearrange("p (t b) -> p t b", b=BS),
                    AF.Identity, bias=b1img[:, gt:gt + 1])

    def scan_step(k, t, layer, gemm0_iter):
        """One LSTM cell step for `layer` at chunk-local step t."""
        if layer == 0:
            cs, xgs, bufW = cs0, xg0[k % 2], bufAB
            wpa, wpb = wp["wp0a"], wp["wp0b"]
        else:
            cs, xgs, bufW = cs1, xg1, bufHAB
            wpa, wpb = wp["wp1a"], wp["wp1b"]

        rAB = bufAB[:].rearrange("p (t q) -> p t q", q=32)
        rHAB = bufHAB[:].rearrange("p (t q) -> p t q", q=32)
        # 1. recurrent (+input for L1) matmuls into 4 psum banks
        for s in range(4):
            for j in range(2):
                dst = gpsum[:, 512 * s + 16 * j: 512 * s + 16 * j + 16]
                sl = slice(32 * s, 32 * s + 12)
                cols = slice(16 * j, 16 * j + 16)
                img = w0img if layer == 0 else w1himg
                buf = rAB if layer == 0 else rHAB
                nc.tensor.matmul(dst, img[sl, :], buf[sl, t, cols],
                                 start=True, stop=True,
                                 tile_position=(32 * s, 0))
        # 2. z = gates_psum + xg  (DVE)
        zslab = work.tile([128, 128], F32, tag="zslab")
        nc.vector.tensor_add(zslab[:], gps_ap(),
                             xgs[:, 128 * t:128 * t + 128])
        # 3. tanh of everything
        tslab = work.tile([128, 128], F32, tag="tslab")
        nc.scalar.activation(tslab[:], zslab[:], AF.Tanh)
        # 4-6. cell update (c~ = 2c)
        aslab = work.tile([128, 32], F32, tag="aslab")
        bslab = work.tile([128, 32], F32, tag="bslab")
        nc.vector.scalar_tensor_tensor(aslab[:], tslab[:, 32:64], 1.0, cs[:],
                                       AOP.add, AOP.mult)
        nc.vector.scalar_tensor_tensor(bslab[:], tslab[:, 0:32], 1.0,
                                       tslab[:, 96:128], AOP.add, AOP.mult)
        nc.vector.scalar_tensor_tensor(cs[:], aslab[:], 0.5, bslab[:],
                                       AOP.mult, AOP.add)
        # 7. th = tanh(c) = tanh(0.5 c~)
        th = work.tile([128, 32], F32, tag="th")
        nc.scalar.activation(th[:], cs[:], AF.Tanh, scale=0.5)
        # 8. u2 = (to+1)*th = 2*o*tanh(c)   (bf16 for the proj matmul)
        u2 = work.tile([128, 32], BF16, tag="u2")
        nc.vector.scalar_tensor_tensor(u2[:], tslab[:, 64:96], 1.0, th[:],
                                       AOP.add, AOP.mult)
        # 9. projection -> ppA (A-pattern) / ppB (B-pattern), one bank each
        for pp, wpx in ((ppA, wpa), (ppB, wpb)):
            for kc in range(2):
                nc.tensor.matmul(pp[:, :],
                                 wpx[:, 128 * kc:128 * kc + 128],
                                 u2[:, 16 * kc:16 * kc + 16],
                                 start=(kc == 0), stop=(kc == 1),
                                 tile_position=(0, 0))
        # 10. h -> bufW col t+1 (bf16)
        rW = rAB if bufW is bufAB else rHAB
        nc.vector.tensor_copy(rW[:, t + 1, 0:16], ppA[:])
        nc.vector.tensor_copy(rW[:, t + 1, 16:32], ppB[:])
        # interleave one GEMM0 group of the next chunk every few steps
        if gemm0_iter is not None and t % 4 == 0:
            g = next(gemm0_iter, None)
            if g is not None:
                g()

    def carry(buf):
        r = buf[:].rearrange("p (t q) -> p t q", q=32)
        nc.vector.tensor_copy(r[:, 0, :], r[:, TC_, :])

    def outputs(k):
        rH = bufHAB[:].rearrange("p (t q) -> p t q", q=32)
        h2 = rH[0:6, 1:TC_ + 1, 0:16]          # h2_t, rows 0-5 (strip0 A)
        nc.vector.tensor_mul(headTc[0:6, :].rearrange("p (t b) -> p t b", b=BS),
                             h2, headTc[0:6, :].rearrange("p (t b) -> p t b", b=BS))
        nhn = max(1, c16 // 512)
        for nh in range(nhn):
            nh_sz = min(512, c16)
            zps = ps_m.tile([1, 512], F32, tag="gps")
            nc.tensor.matmul(zps[0:1, 0:nh_sz],
                             wl8[:, :],
                             headTc[:, nh * nh_sz:(nh + 1) * nh_sz],
                             start=True, stop=True, tile_position=(0, 0))
            nc.scalar.activation(predb[:, nh * nh_sz:(nh + 1) * nh_sz],
                                 zps[0:1, 0:nh_sz], AF.Relu,
                                 bias=blv[:, :])
        nc.sync.dma_start(d["ret6"][:, k * c16:(k + 1) * c16], headTc[0:6, :])
        nc.sync.dma_start(d["pred"][:, k * c16:(k + 1) * c16], predb[:])

    # ---------------- main schedule ----------------
    load_chunk(0)
    for g in gemm0_groups(0):
        g()
    for k in range(nch):
        if k + 1 < nch:
            load_chunk(k + 1)
        nc.sync.dma_start(headTc[:], d["headT"][:, k * c16:(k + 1) * c16])
        g0it = iter(gemm0_groups(k + 1)) if k + 1 < nch else None
        if k > 0:
            carry(bufAB)
            carry(bufHAB)
        for t in range(TC_):
            scan_step(k, t, 0, g0it)
        gemm1(k)
        for t in range(TC_):
            scan_step(k, t, 1, g0it)
        # drain any unemitted gemm0 groups
        if g0it is not None:
            for g in g0it:
                g()
        outputs(k)
    if os.environ.get("KDEBUG"):
        tmp = const.tile([128, (TC_ + 1) * 32], F32, tag="dbgtmp", name="dbgtmp")
        for nm, t_ in [("dbg_xg0", xg0[0]), ("dbg_xg1", xg1),
                       ("dbg_cs0", cs0), ("dbg_cs1", cs1)]:
            nc.sync.dma_start(d[nm][:], t_[:])
        for nm, t_ in [("dbg_bufAB", bufAB), ("dbg_bufHAB", bufHAB)]:
            nc.vector.tensor_copy(tmp[:], t_[:])
            nc.sync.dma_start(d[nm][:], tmp[:])


# --------------------------------------------------------------------------
# host side
# --------------------------------------------------------------------------
_CACHE = {}
LAST_EXEC_NS = None


def _pack_weights(i):
    """Host-side packing of all weight tensors (shared across cores)."""
    f32 = np.float32
    out = {}
    Wb0 = np.concatenate([i["Wih0"], (i["bih0"] + i["bhh0"])[:, None]], 1)
    Wb0 = (Wb0[PERM] * SCALE).astype(f32)            # [1024, 301]
    w0ihT = np.zeros((3, 128, 1024), f32)
    for c in range(3):
        blk = Wb0[:, 128 * c: min(128 * (c + 1), 301)]
        w0ihT[c, 0:blk.shape[1], :] = blk.T
    out["w0ihT"] = w0ihT.transpose(1, 0, 2).reshape(128, 3 * 1024).astype(np.float16)

    out["w0img"] = _strip_img((i["Whh0"][PERM] * SCALE).astype(f32))
    out["w1yimg"] = _strip_img((i["Wih1"][PERM] * SCALE).astype(f32))
    out["w1himg"] = _strip_img((i["Whh1"][PERM] * SCALE).astype(f32))
    out["wp0a"] = _proj_pack(0.5 * i["Whr0"].astype(f32), 0).transpose(1, 0, 2).reshape(128, 256).copy()
    out["wp0b"] = _proj_pack(0.5 * i["Whr0"].astype(f32), 1).transpose(1, 0, 2).reshape(128, 256).copy()
    out["wp1a"] = _proj_pack(0.5 * i["Whr1"].astype(f32), 0).transpose(1, 0, 2).reshape(128, 256).copy()
    out["wp1b"] = _proj_pack(0.5 * i["Whr1"].astype(f32), 1).transpose(1, 0, 2).reshape(128, 256).copy()
    b1 = ((i["bih1"] + i["bhh1"])[PERM, None] * SCALE).astype(f32)  # [1024,1]
    out["b1img"] = b1.reshape(8, 128).T.copy()
    wl = np.zeros((8, 1), f32)
    wl[0:6, 0] = i["Wl"][0, 2:8]
    wl[6:8, 0] = i["Wl"][0, 0:2]
    out["wl8"] = wl
    out["blv"] = np.array([[i["bl"][0]]], f32)
    return out


def kernel(**inputs):
    import ml_dtypes
    i = {k: np.asarray(v) for k, v in inputs.items()}
    if "kernel" in _CACHE:
        nc, dram = _CACHE["kernel"]
    else:
        nc, dram = build_kernel()
        _CACHE["kernel"] = (nc, dram)

    wts = _pack_weights(i)
    for k in ("w0img", "w1yimg", "w1himg", "wp0a", "wp0b", "wp1a", "wp1b"):
        wts[k] = wts[k].astype(np.float16)

    emb = i["embedding"].astype(np.float32)       # [B, T, IN]
    mask = i["mask"].astype(np.float32)           # [T, B, 8]
    feat = i["feature"].astype(np.float32)        # [T, B, 2]

    in_maps = []
    for c in range(NCORE):
        bs = slice(c * BS, (c + 1) * BS)
        xTf = np.zeros((384, TB), np.float32)
        xTf[0:IN] = emb[bs].transpose(2, 1, 0).reshape(IN, TB)
        xTf[300] = 1.0                                # bias ones-row
        headT = np.zeros((8, TB), np.float32)
        headT[0:6] = mask[:, bs, 2:8].transpose(2, 0, 1).reshape(6, TB)
        headT[6:8] = feat[:, bs, :].transpose(2, 0, 1).reshape(2, TB)
        m = {"xT": xTf.reshape(3, 128, TB).transpose(1, 0, 2).reshape(128, 3 * TB).astype(np.float16),
             "headT": headT}
        m.update(wts)
        in_maps.append(m)

    import kernel as _self
    trace = bool(int(os.environ.get("KTRACE", "0")))
    res = run_bass_kernel_spmd(nc, in_maps, core_ids=list(range(NCORE)),
                               trace=trace)
    _self.LAST_EXEC_NS = res.exec_time_ns
    outs = res.results

    pred = np.zeros((T, B, 1), np.float32)
    ret = np.zeros((T, B, 6), np.float32)
    for c in range(NCORE):
        bs = slice(c * BS, (c + 1) * BS)
        r6 = outs[c]["ret6"].reshape(6, T, BS)
        pr = outs[c]["pred"].reshape(T, BS)
        ret[:, bs, :] = r6.transpose(1, 2, 0)
        pred[:, bs, 0] = pr
    lengths = np.asarray(inputs["lengths"])
    return pred, ret, lengths


if __name__ == "__main__":
    import reference
    ins = {k: np.asarray(v) for k, v in reference.setup_inputs().items()}
    outs = kernel(**ins)
    exp = reference.reference(**ins)
    for a, b, nm in zip(outs, exp, ["pred", "ret6", "lengths"]):
        a, b = np.asarray(a, np.float64), np.asarray(b, np.float64)
        err = np.abs(a - b).max()
        scale = max(np.abs(b).max(), 1e-9)
        print(f"{nm}: absmax={err:.3e} rel={err/scale:.3e}")
